# revision 1
# baseline (speedup 1.0000x reference)
"""Causal self-attention on 8 TRN2 NeuronCores.

Problem: B=4, S=2048, D=1024, H=16 heads (hd=64), fp32 in/out.
  qkv = x @ w_qkv + b_qkv ; causal softmax attention ; y @ w_out + b_out

Sharding (tensor-parallel over heads x data-parallel over batch):
  core c -> batch b = c//2, head-group hg = c%2 (8 heads each).
  Each core computes qkv for its 8 heads from x[b], runs attention, and
  produces a partial output  y_local @ w_out[rows]  of shape [S, D].
  Host unshards: out[b] = partial[2b] + partial[2b+1] + b_out.

Device kernel (per core), bf16 matmul operands / fp32 PSUM accumulation:
  - x passed transposed (xT [D, S], bf16) so both projections contract D
    on partitions with no device-side transposes.
  - q,k produced directly transposed (qT/kT [64, S] per head) via
    out = w.T @ x; heads processed in pairs packed at partition offsets
    0-63 / 64-127.  v in natural layout with a ones column (v_aug) so
    the PV matmul also produces the softmax denominator.
  - scores computed transposed (S_T[k, q]) so attT = exp(S_T) is already
    in PV layout; no attention-matrix transposes.  Causal handled by
    block skipping; on diagonal tiles the scores/mask/exp/PV all operate
    only on the live column strip (additive -1e30 mask windows).
  - softmax denominator: ones column accumulates rowsum into row 64 of
    the PV psum; 1/rowsum = exp(-ln(.)) on ACT, DMA partition-shift to
    partition 0, gpsimd partition_broadcast, then the normalization
    multiply fuses into the psum->SBUF eviction of yT.  Max-subtraction
    is skipped (scores are O(1) by construction; exp exact in fp32).
"""

import os
import sys

for _p in ("/root/.axon_site/_ro/trn_rl_repo", "/opt/trn_rl_repo"):
    if os.path.isdir(_p) and _p not in sys.path:
        sys.path.append(_p)

import ml_dtypes
import numpy as np

import concourse.bass as bass  # noqa: F401
import concourse.mybir as mybir
import concourse.tile as tile
from concourse import bacc
from concourse.bass_utils import run_bass_kernel_spmd

B, S, D, H = 4, 2048, 1024, 16
HD = 64
HPC = 8          # heads per core
NPAIR = HPC // 2
KO = D // 128    # contraction chunks over D
ATT_SCALE = 1.0 / np.sqrt(HD)
NEG = -1.0e30

F32 = mybir.dt.float32
F32R = mybir.dt.float32r
BF16 = mybir.dt.bfloat16
NPBF16 = ml_dtypes.bfloat16


def build_nc(S_=S):
    KT = S_ // 128    # k tiles
    TB = S_ // 512    # token blocks for projections

    nc = bacc.Bacc(None)
    xT_d = nc.dram_tensor("xT", [D, S_], BF16, kind="ExternalInput")
    wqk_d = nc.dram_tensor("wqk", [D, NPAIR, 2, 128], BF16, kind="ExternalInput")
    bqk_d = nc.dram_tensor("bqk", [128, NPAIR, 2], F32, kind="ExternalInput")
    wv_d = nc.dram_tensor("wv", [D, HPC * HD], BF16, kind="ExternalInput")
    bv_d = nc.dram_tensor("bv", [128, HPC * HD], F32, kind="ExternalInput")
    wout_d = nc.dram_tensor("wout", [HPC * HD, D], BF16, kind="ExternalInput")
    mask_d = nc.dram_tensor("mask", [128, 896], F32, kind="ExternalInput")
    out_d = nc.dram_tensor("out", [S_, D], F32, kind="ExternalOutput")

    with tile.TileContext(nc) as tc, nc.allow_low_precision("bf16/f32r matmul operands"):
        with (
            tc.tile_pool(name="const", bufs=1) as constp,
            tc.tile_pool(name="psA", bufs=2, space="PSUM") as psA,
            tc.tile_pool(name="psS", bufs=4, space="PSUM") as psS,
            tc.tile_pool(name="psY", bufs=2, space="PSUM") as psY,
        ):
            mask_sb = constp.tile([128, 896], F32)
            nc.sync.dma_start(mask_sb[:], mask_d[:])
            bqk_sb = constp.tile([128, NPAIR, 2], F32)
            nc.sync.dma_start(bqk_sb[:], bqk_d[:])
            bv_sb = constp.tile([128, HPC * HD], F32)
            nc.sync.dma_start(bv_sb[:], bv_d[:])
            # v with ones column (col 64); col 65 is pad
            vaug = constp.tile([128, KT, HPC, 66], BF16)
            nc.gpsimd.memset(vaug[:, :, :, 64], 1.0)
            yT = constp.tile([128, NPAIR, S_], BF16)

            with tc.tile_pool(name="px", bufs=1) as px:
                xT = px.tile([128, KO, S_], BF16)
                xr = xT_d.rearrange("(ko p) t -> p ko t", p=128)
                # k-chunk split: few large contiguous descriptors (a token
                # split was measured much slower despite earlier compute start)
                for i in range(4):
                    nc.sync.dma_start(xT[:, 2 * i : 2 * i + 2, :], xr[:, 2 * i : 2 * i + 2, :])

                # ---- v projection (all heads), biased, into v_aug ----
                with tc.tile_pool(name="pwv", bufs=1) as pwv:
                    wv_sb = pwv.tile([128, KO, HPC * HD], BF16)
                    nc.sync.dma_start(wv_sb[:], wv_d.rearrange("(ko p) c -> p ko c", p=128))
                    for tt in range(KT):
                        ps = psA.tile([128, 512], F32, tag="psA")
                        for k in range(KO):
                            nc.tensor.matmul(
                                ps,
                                xT[:, k, tt * 128 : (tt + 1) * 128],
                                wv_sb[:, k, :],
                                start=(k == 0),
                                stop=(k == KO - 1),
                            )
                        nc.vector.tensor_tensor(
                            vaug[:, tt, :, 0:64],
                            ps[:].rearrange("p (h d) -> p h d", h=HPC),
                            bv_sb[:].rearrange("p (h d) -> p h d", h=HPC),
                            mybir.AluOpType.add,
                        )

                with (
                    tc.tile_pool(name="pqk", bufs=2) as pqk,
                    tc.tile_pool(name="pw", bufs=2) as pw,
                    tc.tile_pool(name="patt", bufs=4) as patt,
                    tc.tile_pool(name="pnorm", bufs=2) as pnorm,
                ):
                    for pr in range(NPAIR):
                        # ---- q/k projection for head pair, packed 64|64 ----
                        wqk_sb = pw.tile([128, KO, 2, 128], BF16, tag="wqk")
                        nc.sync.dma_start(
                            wqk_sb[:],
                            wqk_d.rearrange("(ko p) r c2 c -> p ko r c2 c", p=128)[
                                :, :, pr, :, :
                            ],
                        )
                        qT = pqk.tile([128, S_], BF16, tag="qT")
                        kT = pqk.tile([128, S_], BF16, tag="kT")
                        for cqk in range(2):
                            dst = qT if cqk == 0 else kT
                            for tb0 in range(0, TB, 2):
                                tbs = [tb0] + ([tb0 + 1] if tb0 + 1 < TB else [])
                                pst = [
                                    psA.tile([128, 512], F32, tag="psA", name=f"pj{i}")
                                    for i in range(len(tbs))
                                ]
                                for k in range(KO):
                                    for i, tb in enumerate(tbs):
                                        nc.tensor.matmul(
                                            pst[i],
                                            wqk_sb[:, k, cqk, :],
                                            xT[:, k, tb * 512 : (tb + 1) * 512],
                                            start=(k == 0),
                                            stop=(k == KO - 1),
                                        )
                                for i, tb in enumerate(tbs):
                                    nc.vector.tensor_scalar_add(
                                        dst[:, tb * 512 : (tb + 1) * 512],
                                        pst[i][:],
                                        bqk_sb[:, pr, cqk : cqk + 1],
                                    )

                        # ---- attention for both heads of the pair ----
                        for a in range(S_ // 512):
                            psy = [None, None]
                            for h01 in range(2):
                                psy[h01] = psY.tile(
                                    [65, 512], F32, tag="psY", name=f"psy{h01}"
                                )
                            nj = 4 * a + 4
                            for j in range(nj):
                                o = 128 * j - 512 * a
                                # phase-grouped so the two K=64 score matmuls
                                # sit adjacent in the PE queue and pack onto
                                # disjoint row halves of the array
                                # diagonal tiles: only columns >= o are live;
                                # compute scores/mask/exp on the live strip and
                                # zero the rest of attT
                                oo = max(o, 0)
                                W = 512 - oo
                                pss2, att2 = [], []
                                for h01 in range(2):
                                    lo, hi = h01 * 64, h01 * 64 + 64
                                    pss = psS.tile(
                                        [128, 512], F32, tag="psS", name=f"pss{h01}"
                                    )
                                    nc.tensor.matmul(
                                        pss[:, 0:W],
                                        kT[lo:hi, j * 128 : (j + 1) * 128],
                                        qT[lo:hi, a * 512 + oo : (a + 1) * 512],
                                        start=True,
                                        stop=True,
                                    )
                                    pss2.append(pss)
                                for h01 in range(2):
                                    if o >= 0:
                                        nc.vector.tensor_tensor(
                                            pss2[h01][:, 0:W],
                                            pss2[h01][:, 0:W],
                                            mask_sb[:, 384 : 896 - oo],
                                            mybir.AluOpType.add,
                                        )
                                    att = patt.tile(
                                        [128, 512], BF16, tag="att", name=f"att{h01}"
                                    )
                                    nc.scalar.activation(
                                        att[:, oo:512],
                                        pss2[h01][:, 0:W],
                                        mybir.ActivationFunctionType.Exp,
                                        scale=float(ATT_SCALE),
                                    )
                                    att2.append(att)
                                for h01 in range(2):
                                    # PV restricted to the live strip; PSUM
                                    # has_written bits keep untouched columns
                                    nc.tensor.matmul(
                                        psy[h01][:, oo:512],
                                        vaug[:, j, 2 * pr + h01, 0:65],
                                        att2[h01][:, oo:512],
                                        start=(j == 0),
                                        stop=(j == nj - 1),
                                        skip_group_check=True,
                                    )
                            # ---- normalize + write yT ----
                            for h01 in range(2):
                                # 1/rowsum = exp(-ln(rowsum)) on ACT: costs a
                                # table swap but keeps the 3.3us 1-lane DVE
                                # reciprocal off the in-order DVE (measured
                                # faster than either DVE variant)
                                rtmp = pnorm.tile([65, 512], F32, tag="rt")
                                nc.scalar.activation(
                                    rtmp[64:65, :],
                                    psy[h01][64:65, :],
                                    mybir.ActivationFunctionType.Ln,
                                )
                                nc.scalar.activation(
                                    rtmp[64:65, :],
                                    rtmp[64:65, :],
                                    mybir.ActivationFunctionType.Exp,
                                    scale=-1.0,
                                )
                                rr0 = pnorm.tile([1, 512], F32, tag="rr0")
                                nc.sync.dma_start(rr0[:], rtmp[64:65, :])
                                bc = pnorm.tile([64, 512], F32, tag="bc")
                                nc.gpsimd.partition_broadcast(bc[:], rr0[:])
                                dsts = a * 512
                                if h01 == 0:
                                    nc.vector.tensor_tensor(
                                        yT[0:64, pr, dsts : dsts + 512],
                                        psy[h01][0:64, :],
                                        bc[:],
                                        mybir.AluOpType.mult,
                                    )
                                else:
                                    stg = pnorm.tile([64, 512], BF16, tag="stg")
                                    nc.vector.tensor_tensor(
                                        stg[:],
                                        psy[h01][0:64, :],
                                        bc[:],
                                        mybir.AluOpType.mult,
                                    )
                                    nc.sync.dma_start(
                                        yT[64:128, pr, dsts : dsts + 512], stg[:]
                                    )

            # ---- output projection: partial = yT.T @ w_out ----
            with tc.tile_pool(name="pout", bufs=1) as pout, tc.tile_pool(
                name="postage", bufs=3
            ) as postage:
                wout_sb = pout.tile([128, NPAIR, D], BF16)
                nc.sync.dma_start(wout_sb[:], wout_d.rearrange("(cc p) c -> p cc c", p=128))
                for tt in range(S_ // 128):
                    for nh in range(2):
                        ps = psA.tile([128, 512], F32, tag="psA")
                        for cc in range(NPAIR):
                            nc.tensor.matmul(
                                ps,
                                yT[:, cc, tt * 128 : (tt + 1) * 128],
                                wout_sb[:, cc, nh * 512 : (nh + 1) * 512],
                                start=(cc == 0),
                                stop=(cc == NPAIR - 1),
                            )
                        ot = postage.tile([128, 512], F32, tag="ot")
                        nc.vector.tensor_copy(ot[:], ps[:])
                        nc.sync.dma_start(
                            out_d[tt * 128 : (tt + 1) * 128, nh * 512 : (nh + 1) * 512], ot[:]
                        )

    nc.finalize()
    return nc


def make_host_inputs(x, w_qkv, b_qkv, w_out, b_out, S_=S):
    """Build the 8 per-core input maps (host-side shard/pack/cast)."""
    x = np.asarray(x, dtype=np.float32)
    w_qkv = np.asarray(w_qkv, dtype=np.float32)
    b_qkv = np.asarray(b_qkv, dtype=np.float32)
    w_out = np.asarray(w_out, dtype=np.float32)

    mask = np.where(
        np.arange(896)[None, :] >= np.arange(128)[:, None] + 384, 0.0, NEG
    ).astype(np.float32)

    per_hg = {}
    for hg in range(2):
        wqk = np.empty((D, NPAIR, 2, 128), np.float32)
        bqk = np.empty((128, NPAIR, 2), np.float32)
        for p in range(NPAIR):
            h0, h1 = hg * HPC + 2 * p, hg * HPC + 2 * p + 1
            wqk[:, p, 0, 0:64] = w_qkv[:, h0 * HD : (h0 + 1) * HD]
            wqk[:, p, 0, 64:128] = w_qkv[:, h1 * HD : (h1 + 1) * HD]
            wqk[:, p, 1, 0:64] = w_qkv[:, D + h0 * HD : D + (h0 + 1) * HD]
            wqk[:, p, 1, 64:128] = w_qkv[:, D + h1 * HD : D + (h1 + 1) * HD]
            bqk[0:64, p, 0] = b_qkv[h0 * HD : (h0 + 1) * HD]
            bqk[64:128, p, 0] = b_qkv[h1 * HD : (h1 + 1) * HD]
            bqk[0:64, p, 1] = b_qkv[D + h0 * HD : D + (h0 + 1) * HD]
            bqk[64:128, p, 1] = b_qkv[D + h1 * HD : D + (h1 + 1) * HD]
        wv = w_qkv[:, 2 * D + hg * 512 : 2 * D + (hg + 1) * 512]
        bv = np.broadcast_to(
            b_qkv[2 * D + hg * 512 : 2 * D + (hg + 1) * 512], (128, 512)
        ).copy()
        wout = w_out[hg * 512 : (hg + 1) * 512, :]
        per_hg[hg] = dict(
            wqk=np.ascontiguousarray(wqk.astype(NPBF16)),
            bqk=bqk,
            wv=np.ascontiguousarray(wv.astype(NPBF16)),
            bv=bv,
            wout=np.ascontiguousarray(wout.astype(NPBF16)),
        )

    xT_by_b = [
        np.ascontiguousarray(x[b, :S_].T.astype(NPBF16)) for b in range(B)
    ]
    in_maps = []
    for c in range(8):
        b, hg = c // 2, c % 2
        m = dict(per_hg[hg])
        m["xT"] = xT_by_b[b]
        m["mask"] = mask
        in_maps.append(m)
    return in_maps


_NC_CACHE = {}


def _get_nc(S_=S):
    if S_ not in _NC_CACHE:
        _NC_CACHE[S_] = build_nc(S_)
    return _NC_CACHE[S_]


def kernel(x, w_qkv, b_qkv, w_out, b_out):
    x = np.asarray(x, dtype=np.float32)
    b_out = np.asarray(b_out, dtype=np.float32)
    in_maps = make_host_inputs(x, w_qkv, b_qkv, w_out, b_out)
    nc = _get_nc()
    res = run_bass_kernel_spmd(nc, in_maps, list(range(8))).results
    out = np.empty((B, S, D), np.float32)
    for b in range(B):
        out[b] = res[2 * b]["out"] + res[2 * b + 1]["out"] + b_out[None, :]
    return out



# revision 16
# speedup vs baseline: 1.1856x; 1.1856x over previous
"""Causal self-attention on 8 TRN2 NeuronCores.

Problem: B=4, S=2048, D=1024, H=16 heads (hd=64), fp32 in/out.
  qkv = x @ w_qkv + b_qkv ; causal softmax attention ; y @ w_out + b_out

Sharding (tensor-parallel over heads x data-parallel over batch):
  core c -> batch b = c//2, head-group hg = c%2 (8 heads each).
  Each core computes qkv for its 8 heads from x[b], runs attention, and
  produces a partial output  y_local @ w_out[rows]  of shape [S, D].
  Host unshards: out[b] = partial[2b] + partial[2b+1] + b_out.

Device kernel (per core), bf16 matmul operands / fp32 PSUM accumulation.
v2 layout notes (changes vs the first working kernel):
  - The attention inner phase is ACT-bound (exp of the score tiles), so
    the projection matmuls for the NEXT head pair and the v projection
    are interleaved into the attention issue stream: the PE works on
    projections while ACT digests exps.
  - Scores accumulate into [128, 1024] PSUM tiles (two 128-key tiles per
    bank pair) so exps batch into half as many ACTIVATE instructions
    (the ACT fixed overhead per instruction is ~200ns).
  - Softmax normalization is deferred out of the inner loop: psy evicts
    store UNNORMALIZED yT plus the rowsum row (via the ones column of
    v_aug).  Per pair, a single custom-DVE reciprocal_approx_fast
    computes 1/rowsum for all 8 (h01, a) rows at once — no Ln/Exp
    activation-table ping-pong (the table reloads alone cost ~42us on
    ScalarE in v1).  gpsimd broadcasts build a [128, S] recip tensor and
    one 4x-mode DVE multiply normalizes the pair's yT slab, all issued
    during the NEXT pair's attention so it hides under exp time.
  - Startup: xT is DMAd in 8 k-chunks and the pair-0 q/k projection
    runs k-major trailing the DMA, so the PE starts ~2us in instead of
    waiting ~28us for the full xT + wv load.
"""

import os
import sys

for _p in ("/root/.axon_site/_ro/trn_rl_repo", "/opt/trn_rl_repo"):
    if os.path.isdir(_p) and _p not in sys.path:
        sys.path.append(_p)

import ml_dtypes
import numpy as np

import concourse.bass as bass  # noqa: F401
import concourse.mybir as mybir
import concourse.tile as tile
from concourse import bacc
from concourse.bass_utils import run_bass_kernel_spmd

B, S, D, H = 4, 2048, 1024, 16
HD = 64
HPC = 8          # heads per core
NPAIR = HPC // 2
KO = D // 128    # contraction chunks over D
ATT_SCALE = 1.0 / np.sqrt(HD)
NEG = -1.0e30

F32 = mybir.dt.float32
F32R = mybir.dt.float32r
BF16 = mybir.dt.bfloat16
NPBF16 = ml_dtypes.bfloat16


def build_nc(S_=S, debug=False):
    KT = S_ // 128    # k tiles (128-token)
    TB = S_ // 512    # token blocks (512-token) == attention q-blocks

    nc = bacc.Bacc(None)
    if debug:
        rs_dbg = nc.dram_tensor("rs_dbg", [8, NPAIR, 512], F32, kind="ExternalOutput")
        rcp_dbg = nc.dram_tensor("rcp_dbg", [8, NPAIR, 512], F32, kind="ExternalOutput")
        rb_dbg = nc.dram_tensor("rb_dbg", [128, NPAIR, S], F32, kind="ExternalOutput")
        yT_dbg = nc.dram_tensor("yT_dbg", [128, NPAIR, S], BF16, kind="ExternalOutput")
        yN_dbg = nc.dram_tensor("yN_dbg", [128, NPAIR, S], BF16, kind="ExternalOutput")
    xT_d = nc.dram_tensor("xT", [D, S_], BF16, kind="ExternalInput")
    wqk_d = nc.dram_tensor("wqk", [D, NPAIR, 2, 128], BF16, kind="ExternalInput")
    bqk_d = nc.dram_tensor("bqk", [128, NPAIR, 2], F32, kind="ExternalInput")
    wv_d = nc.dram_tensor("wv", [D, HPC * HD], BF16, kind="ExternalInput")
    bv_d = nc.dram_tensor("bv", [128, HPC * HD], F32, kind="ExternalInput")
    wout_d = nc.dram_tensor("wout", [HPC * HD, D], BF16, kind="ExternalInput")
    mask_d = nc.dram_tensor("mask", [128, 896], F32, kind="ExternalInput")
    out_d = nc.dram_tensor("out", [S_, D], F32, kind="ExternalOutput")

    wqk_r = wqk_d.rearrange("(ko p) r c2 c -> p ko r c2 c", p=128)

    with tile.TileContext(nc) as tc, nc.allow_low_precision("bf16/f32r matmul operands"):
        with (
            tc.tile_pool(name="const", bufs=1) as constp,
            tc.tile_pool(name="psS", bufs=3, space="PSUM") as psS,
            tc.tile_pool(name="psY", bufs=2, space="PSUM") as psY,
        ):
            bqk_sb = constp.tile([128, NPAIR, 2], F32)
            nc.sync.dma_start(bqk_sb[:], bqk_d[:])
            mask_sb = constp.tile([128, 896], F32)
            bv_sb = constp.tile([128, HPC * HD], F32)
            # v with ones column (col 64) -> PV matmul also emits rowsum
            vaug = constp.tile([128, KT, HPC, 66], BF16)
            nc.gpsimd.memset(vaug[:, :, :, 64], 1.0)
            yT = constp.tile([128, NPAIR, S_], BF16)    # unnormalized
            yN = constp.tile([128, NPAIR, S_], BF16)    # normalized
            # rowsums: partition = 2*a + h01, free idx = pair (keeps every
            # slice at partition offset 0 — custom-DVE ops require it)
            rs = constp.tile([8, NPAIR, 512], F32)
            rcp32 = constp.tile([8, NPAIR, 512], F32)

            with (
                tc.tile_pool(name="px", bufs=1) as px,
                tc.tile_pool(name="pwv", bufs=1) as pwv,
                tc.tile_pool(name="pw", bufs=2) as pw,
                tc.tile_pool(name="pqk", bufs=2) as pqk,
                tc.tile_pool(name="patt", bufs=4) as patt,
                tc.tile_pool(name="pnorm", bufs=2) as pnorm,
            ):
                xT = px.tile([128, KO, S_], BF16)
                xr = xT_d.rearrange("(ko p) t -> p ko t", p=128)
                wqk_sb0 = pw.tile([128, KO, 2, 128], BF16, tag="wqk")
                nc.sync.dma_start(wqk_sb0[:], wqk_r[:, :, 0, :, :])
                # x in 8 chunk DMAs so the k-major prologue trails the DMA
                for k in range(KO):
                    nc.sync.dma_start(xT[:, k, :], xr[:, k, :])
                wv_sb = pwv.tile([128, KO, HPC * HD], BF16)
                nc.sync.dma_start(wv_sb[:], wv_d.rearrange("(ko p) c -> p ko c", p=128))
                nc.sync.dma_start(bv_sb[:], bv_d[:])
                nc.sync.dma_start(mask_sb[:], mask_d[:])

                qkt = {}
                for pr in range(NPAIR):
                    qkt[pr] = (
                        pqk.tile([128, S_], BF16, tag="qT", name=f"qT{pr}"),
                        pqk.tile([128, S_], BF16, tag="kT", name=f"kT{pr}"),
                    )

                def qk_evict(pr, tb, ps_q, ps_k):
                    qT, kT = qkt[pr]
                    nc.vector.tensor_scalar_add(
                        qT[:, tb * 512 : (tb + 1) * 512], ps_q, bqk_sb[:, pr, 0:1]
                    )
                    nc.vector.tensor_scalar_add(
                        kT[:, tb * 512 : (tb + 1) * 512], ps_k, bqk_sb[:, pr, 1:2]
                    )

                # ---- prologue: q/k projection for pair 0, k-major ----
                for tbp in ((0, 1), (2, 3)):
                    pp = [
                        psS.tile([128, 1024], F32, tag="psS", name=f"pp{i}")
                        for i in range(2)
                    ]
                    for k in range(KO):
                        for cqk in range(2):
                            for i, tb in enumerate(tbp):
                                nc.tensor.matmul(
                                    pp[i][:, cqk * 512 : (cqk + 1) * 512],
                                    wqk_sb0[:, k, cqk, :],
                                    xT[:, k, tb * 512 : (tb + 1) * 512],
                                    start=(k == 0),
                                    stop=(k == KO - 1),
                                    skip_group_check=True,
                                )
                    for i, tb in enumerate(tbp):
                        qk_evict(0, tb, pp[i][:, 0:512], pp[i][:, 512:1024])

                def qk_chunk(pr, tb, wqk_sb):
                    """projection chunk for pair pr, token block tb (q and k)."""
                    ps = psS.tile([128, 1024], F32, tag="psS", name="pj")
                    for k in range(KO):
                        for cqk in range(2):
                            nc.tensor.matmul(
                                ps[:, cqk * 512 : (cqk + 1) * 512],
                                wqk_sb[:, k, cqk, :],
                                xT[:, k, tb * 512 : (tb + 1) * 512],
                                start=(k == 0),
                                stop=(k == KO - 1),
                                skip_group_check=True,
                            )
                    qk_evict(pr, tb, ps[:, 0:512], ps[:, 512:1024])

                def v_chunk(a):
                    """v projection for token tiles 4a..4a+3 (all heads)."""
                    for pi in range(2):
                        ps = psS.tile([128, 1024], F32, tag="psS", name="pv")
                        for k in range(KO):
                            for h in range(2):
                                tt = 4 * a + 2 * pi + h
                                nc.tensor.matmul(
                                    ps[:, h * 512 : (h + 1) * 512],
                                    xT[:, k, tt * 128 : (tt + 1) * 128],
                                    wv_sb[:, k, :],
                                    start=(k == 0),
                                    stop=(k == KO - 1),
                                    skip_group_check=True,
                                )
                        for h in range(2):
                            tt = 4 * a + 2 * pi + h
                            nc.vector.tensor_tensor(
                                vaug[:, tt, :, 0:64],
                                ps[:, h * 512 : (h + 1) * 512].rearrange(
                                    "p (h d) -> p h d", h=HPC
                                ),
                                bv_sb[:].rearrange("p (h d) -> p h d", h=HPC),
                                mybir.AluOpType.add,
                            )

                def norm_pair(pr):
                    """1/rowsum for all 8 rows of pair pr, broadcast, and
                    normalize the pair's yT slab into yN.  Issued during the
                    following pair's attention so it hides under exp time."""
                    nc.vector.reciprocal_approx_fast(rcp32[:, pr, :], rs[:, pr, :])
                    # gpsimd partition_broadcast only works in the exact
                    # full-tile fp32 [1,512]->[64,512] form (offset APs give
                    # garbage on hardware), so stage each row through a
                    # dedicated tile; the h1 half partition-shifts via DMA.
                    rb = pnorm.tile([128, S_], F32, tag="rb")
                    for a in range(TB):
                        for h01 in range(2):
                            rr = pnorm.tile([1, 512], F32, tag="rr")
                            nc.sync.dma_start(
                                rr[:], rcp32[2 * a + h01 : 2 * a + h01 + 1, pr, :]
                            )
                            bc = pnorm.tile([64, 512], F32, tag="bc")
                            nc.gpsimd.partition_broadcast(bc[:], rr[:])
                            if h01 == 0:
                                nc.vector.tensor_copy(
                                    rb[0:64, a * 512 : (a + 1) * 512], bc[:]
                                )
                            else:
                                nc.sync.dma_start(
                                    rb[64:128, a * 512 : (a + 1) * 512], bc[:]
                                )
                    nc.vector.tensor_tensor(
                        yN[:, pr, :], yT[:, pr, :], rb[:], mybir.AluOpType.mult
                    )
                    if debug:
                        nc.sync.dma_start(rb_dbg[:, pr, :], rb[:])

                def attn_block(pr, a):
                    qT, kT = qkt[pr]
                    nj = 4 * a + 4
                    psy = [
                        psY.tile([65, 512], F32, tag="psY", name=f"psy{h}")
                        for h in range(2)
                    ]
                    for jb in range(nj // 2):
                        js = (2 * jb, 2 * jb + 1)
                        diag = jb >= 2 * a
                        pss = [
                            psS.tile([128, 1024], F32, tag="psS", name=f"pss{h}")
                            for h in range(2)
                        ]
                        # scores: head pair adjacent so the two K=64 matmuls
                        # pack onto disjoint row halves of the PE array
                        for ii, j in enumerate(js):
                            oo = max(128 * j - 512 * a, 0)
                            for h01 in range(2):
                                lo, hi = h01 * 64, h01 * 64 + 64
                                nc.tensor.matmul(
                                    pss[h01][:, ii * 512 + oo : (ii + 1) * 512],
                                    kT[lo:hi, j * 128 : (j + 1) * 128],
                                    qT[lo:hi, a * 512 + oo : (a + 1) * 512],
                                    start=True,
                                    stop=True,
                                    skip_group_check=True,
                                )
                        att2 = []
                        for h01 in range(2):
                            att = patt.tile([128, 1024], BF16, tag="att", name=f"att{h01}")
                            if not diag:
                                nc.scalar.activation(
                                    att[:],
                                    pss[h01][:],
                                    mybir.ActivationFunctionType.Exp,
                                    scale=float(ATT_SCALE),
                                )
                            else:
                                for ii, j in enumerate(js):
                                    oo = 128 * j - 512 * a
                                    nc.vector.tensor_tensor(
                                        pss[h01][:, ii * 512 + oo : (ii + 1) * 512],
                                        pss[h01][:, ii * 512 + oo : (ii + 1) * 512],
                                        mask_sb[:, 384 : 896 - oo],
                                        mybir.AluOpType.add,
                                    )
                                    nc.scalar.activation(
                                        att[:, ii * 512 + oo : (ii + 1) * 512],
                                        pss[h01][:, ii * 512 + oo : (ii + 1) * 512],
                                        mybir.ActivationFunctionType.Exp,
                                        scale=float(ATT_SCALE),
                                    )
                            att2.append(att)
                        for h01 in range(2):
                            for ii, j in enumerate(js):
                                oo = max(128 * j - 512 * a, 0)
                                nc.tensor.matmul(
                                    psy[h01][:, oo:512],
                                    vaug[:, j, 2 * pr + h01, 0:65],
                                    att2[h01][:, ii * 512 + oo : (ii + 1) * 512],
                                    start=(j == 0),
                                    stop=(j == nj - 1),
                                    skip_group_check=True,
                                )
                    # evict: unnormalized y + rowsum rows (rowsums staged through
                    # SBUF — DMA cannot read PSUM — then one DMA drops both
                    # onto adjacent rs rows r = 8*pr + 2*a + h01)
                    srow = pnorm.tile([65, 2, 512], F32, tag="srow")
                    for h01 in range(2):
                        nc.vector.tensor_copy(srow[64:65, h01, :], psy[h01][64:65, :])
                        if h01 == 0:
                            nc.vector.tensor_copy(
                                yT[0:64, pr, a * 512 : (a + 1) * 512], psy[0][0:64, :]
                            )
                        else:
                            stg = pnorm.tile([64, 512], BF16, tag="stg")
                            nc.vector.tensor_copy(stg[:], psy[1][0:64, :])
                            nc.sync.dma_start(
                                yT[64:128, pr, a * 512 : (a + 1) * 512], stg[:]
                            )
                    nc.sync.dma_start(
                        rs[2 * a : 2 * a + 2, pr, :], srow[64:65, :, :]
                    )

                # ---- main loop: attention(pr) with qkproj(pr+1) + vproj
                # interleaved ----
                wqk_next = None
                for pr in range(NPAIR):
                    if pr > 0:
                        norm_pair(pr - 1)
                    for a in range(TB):
                        if pr == 0:
                            v_chunk(a)
                        if pr < NPAIR - 1:
                            if a == 0:
                                wqk_next = pw.tile(
                                    [128, KO, 2, 128], BF16, tag="wqk", name="wqkn"
                                )
                                nc.sync.dma_start(
                                    wqk_next[:], wqk_r[:, :, pr + 1, :, :]
                                )
                            qk_chunk(pr + 1, a, wqk_next)
                        attn_block(pr, a)
                norm_pair(NPAIR - 1)
                if debug:
                    nc.sync.dma_start(rs_dbg[:], rs[:])
                    nc.sync.dma_start(rcp_dbg[:], rcp32[:])
                    nc.sync.dma_start(yT_dbg[:], yT[:])
                    nc.sync.dma_start(yN_dbg[:], yN[:])

            # ---- output projection: partial = yN.T @ w_out ----
            with tc.tile_pool(name="pout", bufs=1) as pout, tc.tile_pool(
                name="postage", bufs=3
            ) as postage:
                wout_sb = pout.tile([128, NPAIR, D], BF16)
                nc.sync.dma_start(wout_sb[:], wout_d.rearrange("(cc p) c -> p cc c", p=128))
                for tt in range(KT):
                    ps = psS.tile([128, 1024], F32, tag="psS", name="po")
                    for nh in range(2):
                        for cc in range(NPAIR):
                            nc.tensor.matmul(
                                ps[:, nh * 512 : (nh + 1) * 512],
                                yN[:, cc, tt * 128 : (tt + 1) * 128],
                                wout_sb[:, cc, nh * 512 : (nh + 1) * 512],
                                start=(cc == 0),
                                stop=(cc == NPAIR - 1),
                                skip_group_check=True,
                            )
                    ot = postage.tile([128, 1024], F32, tag="ot")
                    nc.vector.tensor_copy(ot[:], ps[:])
                    nc.sync.dma_start(out_d[tt * 128 : (tt + 1) * 128, :], ot[:])

    nc.finalize()
    return nc


def make_host_inputs(x, w_qkv, b_qkv, w_out, b_out, S_=S):
    """Build the 8 per-core input maps (host-side shard/pack/cast)."""
    x = np.asarray(x, dtype=np.float32)
    w_qkv = np.asarray(w_qkv, dtype=np.float32)
    b_qkv = np.asarray(b_qkv, dtype=np.float32)
    w_out = np.asarray(w_out, dtype=np.float32)

    mask = np.where(
        np.arange(896)[None, :] >= np.arange(128)[:, None] + 384, 0.0, NEG
    ).astype(np.float32)

    per_hg = {}
    for hg in range(2):
        wqk = np.empty((D, NPAIR, 2, 128), np.float32)
        bqk = np.empty((128, NPAIR, 2), np.float32)
        for p in range(NPAIR):
            h0, h1 = hg * HPC + 2 * p, hg * HPC + 2 * p + 1
            wqk[:, p, 0, 0:64] = w_qkv[:, h0 * HD : (h0 + 1) * HD]
            wqk[:, p, 0, 64:128] = w_qkv[:, h1 * HD : (h1 + 1) * HD]
            wqk[:, p, 1, 0:64] = w_qkv[:, D + h0 * HD : D + (h0 + 1) * HD]
            wqk[:, p, 1, 64:128] = w_qkv[:, D + h1 * HD : D + (h1 + 1) * HD]
            bqk[0:64, p, 0] = b_qkv[h0 * HD : (h0 + 1) * HD]
            bqk[64:128, p, 0] = b_qkv[h1 * HD : (h1 + 1) * HD]
            bqk[0:64, p, 1] = b_qkv[D + h0 * HD : D + (h0 + 1) * HD]
            bqk[64:128, p, 1] = b_qkv[D + h1 * HD : D + (h1 + 1) * HD]
        wv = w_qkv[:, 2 * D + hg * 512 : 2 * D + (hg + 1) * 512]
        bv = np.broadcast_to(
            b_qkv[2 * D + hg * 512 : 2 * D + (hg + 1) * 512], (128, 512)
        ).copy()
        wout = w_out[hg * 512 : (hg + 1) * 512, :]
        per_hg[hg] = dict(
            wqk=np.ascontiguousarray(wqk.astype(NPBF16)),
            bqk=bqk,
            wv=np.ascontiguousarray(wv.astype(NPBF16)),
            bv=bv,
            wout=np.ascontiguousarray(wout.astype(NPBF16)),
        )

    xT_by_b = [
        np.ascontiguousarray(x[b, :S_].T.astype(NPBF16)) for b in range(B)
    ]
    in_maps = []
    for c in range(8):
        b, hg = c // 2, c % 2
        m = dict(per_hg[hg])
        m["xT"] = xT_by_b[b]
        m["mask"] = mask
        in_maps.append(m)
    return in_maps


_NC_CACHE = {}


def _get_nc(S_=S):
    if S_ not in _NC_CACHE:
        _NC_CACHE[S_] = build_nc(S_)
    return _NC_CACHE[S_]


def kernel(x, w_qkv, b_qkv, w_out, b_out):
    x = np.asarray(x, dtype=np.float32)
    b_out = np.asarray(b_out, dtype=np.float32)
    in_maps = make_host_inputs(x, w_qkv, b_qkv, w_out, b_out)
    nc = _get_nc()
    res = run_bass_kernel_spmd(nc, in_maps, list(range(8))).results
    out = np.empty((B, S, D), np.float32)
    for b in range(B):
        out[b] = res[2 * b]["out"] + res[2 * b + 1]["out"] + b_out[None, :]
    return out
